# revision 40
# baseline (speedup 1.0000x reference)
"""Multi-head causal attention on 8 TRN2 NeuronCores.

Sharding: 2-way data parallel (batch) x 4-way tensor parallel (heads).
Core c handles batch c//4 and heads (c%4)*4 .. (c%4)*4+3.  Each core
computes q/k/v projections for its 4 heads (column-sharded QKV weights),
causal attention for those heads, and the row-sharded slice of the output
projection, producing a full-shape partial output for its batch.  Host sums
the 4 partials per batch and adds bo + bv @ Wo.T (the per-head value bias
commutes through the output projection because attention rows sum to 1).

All matmuls run in bf16 (full PE rate at any moving width; fp32 PSUM
accumulate).  Layout/structure choices:
  - x is passed transposed (E-major) bf16 so QKV matmuls need no on-device
    transposes; weights are host-transposed likewise
  - scores are computed transposed [k, q] so the attn @ v matmul's operands
    arrive in exactly the layout the PE wants; heads are processed in PAIRS
    with a [128, 2, 512] PSUM score tile so one ACT exp call covers both
  - softmax denominators never touch the PE: exp tiles are accumulated over
    k-tiles with cheap 2x-mode bf16 DVE adds (per-partition partial sums,
    <=16 terms each, so bf16 rounding averages out across the 128-partition
    fp32 gpsimd partition_all_reduce), then one DVE divide normalizes
  - diagonal k-tiles only compute/accumulate their causally valid column
    sub-range (exact: diagonal tiles are last in each k-loop)
  - output projection is fused after each q-tile; evacuation copies rotate
    over DVE/ACT/GPSIMD; PSUM budget: proj+out-proj 2 banks, paired scores
    2x2 banks, attention accumulators 2 banks
"""

import sys

if "/opt/trn_rl_repo" not in sys.path:
    sys.path.insert(0, "/opt/trn_rl_repo")

import numpy as np

import concourse.bass as bass  # noqa: F401  (engine namespaces live on nc)
import concourse.tile as tile
from concourse import bacc, bass_isa, mybir
from concourse.bass_utils import run_bass_kernel_spmd

F32 = mybir.dt.float32
BF16 = mybir.dt.bfloat16
AF = mybir.ActivationFunctionType
ALU = mybir.AluOpType

B, S, E = 2, 2048, 2048
H, D = 16, 128
NCORES = 8
HPC = 4                    # heads per core
M = HPC * D                # local qkv channels per core = 512
EO = E // 128              # 16 contraction chunks
XT = 512                   # token-tile width for projections
NT = S // XT               # 4 token tiles
QT = 512                   # q-tile width for attention
NQT = S // QT              # 4 q-tiles
ET = 512                   # e-tile width for out-projection
SCALE = 1.0 / float(np.sqrt(D))
MASK_BIAS = -30.0


def build_nc():
    nc = bacc.Bacc(trn_type="TRN2", target_bir_lowering=False, num_swdge_queues=4)

    xT = nc.declare_dram_parameter("xT", [E, S], BF16, isOutput=False)
    wq = nc.declare_dram_parameter("wq", [E, M], BF16, isOutput=False)
    wk = nc.declare_dram_parameter("wk", [E, M], BF16, isOutput=False)
    wv = nc.declare_dram_parameter("wv", [E, M], BF16, isOutput=False)
    wo = nc.declare_dram_parameter("wo", [M, E], BF16, isOutput=False)
    bq = nc.declare_dram_parameter("bq", [128, HPC], F32, isOutput=False)
    bk = nc.declare_dram_parameter("bk", [128, HPC], F32, isOutput=False)
    tb = nc.declare_dram_parameter("tb", [128, 128], F32, isOutput=False)
    o = nc.declare_dram_parameter("o", [S, E], BF16, isOutput=True)

    with tile.TileContext(nc) as tc:
        _body(tc, nc, xT, wq, wk, wv, wo, bq, bk, tb, o)
    nc.compile()
    return nc


def _body(tc, nc, xT, wq, wk, wv, wo, bq, bk, tb, o):
    from contextlib import ExitStack

    ctx = ExitStack()
    with ctx:
        # PSUM pools first so the paired score tile lands bank-aligned.
        psS = ctx.enter_context(tc.tile_pool(name="psS", bufs=2, space="PSUM"))
        psA = ctx.enter_context(tc.tile_pool(name="psA", bufs=2, space="PSUM"))
        psU = ctx.enter_context(tc.tile_pool(name="psU", bufs=2, space="PSUM"))
        wpool = ctx.enter_context(tc.tile_pool(name="w", bufs=1))
        xpool = ctx.enter_context(tc.tile_pool(name="x", bufs=2))
        qkv = ctx.enter_context(tc.tile_pool(name="qkv", bufs=1))
        otp = ctx.enter_context(tc.tile_pool(name="ot", bufs=1))
        epool = ctx.enter_context(tc.tile_pool(name="e", bufs=6))
        eap = ctx.enter_context(tc.tile_pool(name="ea", bufs=4))
        dpool = ctx.enter_context(tc.tile_pool(name="dn", bufs=3))
        osp = ctx.enter_context(tc.tile_pool(name="os", bufs=8))

        # ---- weights / constants ----
        wq_sb = wpool.tile([128, EO, M], BF16, tag="wq")
        wk_sb = wpool.tile([128, EO, M], BF16, tag="wk")
        wv_sb = wpool.tile([128, EO, M], BF16, tag="wv")
        wo_sb = wpool.tile([128, HPC, E], BF16, tag="wo")

        # Warm the PE (HAM clock gate) with tiny matmuls while x0/weights
        # stream in; the warm tiles come from memsets so they have no DMA
        # dependency.
        warm_l = wpool.tile([128, 1], BF16, tag="wl")
        warm_r = wpool.tile([128, 512], BF16, tag="wr")
        nc.vector.memset(warm_l[:], 0.0)
        nc.vector.memset(warm_r[:], 0.0)
        warm = psS.tile([128, 2, 512], F32, tag="sc")
        for _ in range(18):
            nc.tensor.matmul(warm[:1, 0, :], warm_l[:], warm_r[:],
                             start=True, stop=True)

        # x tile 0 and the first head-pair's wq/wk stream in first so the
        # first projection groups start as soon as possible; wv is needed by
        # the v-projection at the end of tile 0, wo only at the first
        # out-projection.
        _wqr = wq.rearrange("(eo p) m -> p eo m", p=128)
        _wkr = wk.rearrange("(eo p) m -> p eo m", p=128)
        _xr = xT.rearrange("(eo p) s -> p eo s", p=128)
        x_first = xpool.tile([128, EO, 512], BF16, tag="x")
        nc.sync.dma_start(x_first[:, 0:8], _xr[:, 0:8, 0:512])
        nc.sync.dma_start(wq_sb[:, :, 0:256], _wqr[:, :, 0:256])
        nc.gpsimd.dma_start(x_first[:, 8:16], _xr[:, 8:16, 0:512])
        nc.gpsimd.dma_start(wk_sb[:, :, 0:256], _wkr[:, :, 0:256])
        nc.gpsimd.dma_start(wq_sb[:, :, 256:512], _wqr[:, :, 256:512])
        nc.gpsimd.dma_start(wk_sb[:, :, 256:512], _wkr[:, :, 256:512])
        nc.gpsimd.dma_start(wv_sb[:], wv.rearrange("(eo p) m -> p eo m", p=128))
        x_second = xpool.tile([128, EO, 512], BF16, tag="x")
        nc.gpsimd.dma_start(x_second[:], _xr[:, :, 512:1024])
        nc.gpsimd.dma_start(wo_sb[:], wo.rearrange("(h p) e -> p h e", p=128))

        bq_sb = wpool.tile([128, HPC], F32, tag="bq")
        bk_sb = wpool.tile([128, HPC], F32, tag="bk")
        tb_sb = wpool.tile([128, 128], F32, tag="tb")
        nc.sync.dma_start(bq_sb[:], bq[:])
        nc.sync.dma_start(bk_sb[:], bk[:])
        nc.sync.dma_start(tb_sb[:], tb[:])

        qT_sb = qkv.tile([128, HPC, S], BF16, tag="qT")
        kT_sb = qkv.tile([128, HPC, S], BF16, tag="kT")
        v_sb = qkv.tile([128, S // 128, M], BF16, tag="v")
        oT_sb = otp.tile([128, HPC, S], BF16, tag="oT")

        def proj_group(off, wdt, x_t, h, w_sb, dst, bias, scl):
            ps = psA.tile([128, 512], F32, tag="qkv")
            for eo in range(EO):
                nc.tensor.matmul(
                    ps[:, :wdt],
                    w_sb[:, eo, h * D:(h + 1) * D],
                    x_t[:, eo, :wdt],
                    start=(eo == 0),
                    stop=(eo == EO - 1),
                )
            nc.scalar.activation(
                dst[:, h, off:off + wdt],
                ps[:, :wdt],
                AF.Identity,
                bias=bias[:, h:h + 1],
                scale=scl,
            )

        def proj_tile(off, wdt, x_t):
            # head-pair-major order so the first groups only need the first
            # half of wq/wk
            for hp in (0, 1):
                for h in (2 * hp, 2 * hp + 1):
                    proj_group(off, wdt, x_t, h, wq_sb, qT_sb, bq_sb, SCALE)
                for h in (2 * hp, 2 * hp + 1):
                    proj_group(off, wdt, x_t, h, wk_sb, kT_sb, bk_sb, 1.0)
            for st in range(wdt // 128):
                ps = psA.tile([128, 512], F32, tag="qkv")
                for eo in range(EO):
                    nc.tensor.matmul(
                        ps[:],
                        x_t[:, eo, st * 128:(st + 1) * 128],
                        wv_sb[:, eo, :],
                        start=(eo == 0),
                        stop=(eo == EO - 1),
                    )
                nc.scalar.copy(v_sb[:, off // 128 + st, :], ps[:])

        def attn_pair(hp, qt):
            ha, hb = 2 * hp, 2 * hp + 1
            q_a = qT_sb[:, ha, qt * QT:(qt + 1) * QT]
            q_b = qT_sb[:, hb, qt * QT:(qt + 1) * QT]
            ut_a = psU.tile([128, 512], F32, tag="ut")
            ut_b = psU.tile([128, 512], F32, tag="ut")
            ea_a = eap.tile([128, 512], BF16, tag="ea")
            ea_b = eap.tile([128, 512], BF16, tag="ea")
            nkt = (qt + 1) * (QT // 128)
            for kt in range(nkt):
                jj = kt - qt * (QT // 128)
                # columns < jj*128 of this k-tile's block are causally
                # masked; diagonal tiles come last in the k-loop, so
                # accumulating only the valid sub-range is exact.
                lo = max(jj, 0) * 128
                sc = psS.tile([128, 2, 512], F32, tag="sc")
                for j, q_h, hh in ((0, q_a, ha), (1, q_b, hb)):
                    nc.tensor.matmul(
                        sc[:, j, lo:],
                        kT_sb[:, hh, kt * 128:(kt + 1) * 128],
                        q_h[:, lo:],
                        start=True,
                        stop=True,
                    )
                if jj >= 0:
                    nc.vector.tensor_tensor(
                        sc[:, 0, lo:lo + 128], sc[:, 0, lo:lo + 128],
                        tb_sb[:], ALU.add,
                    )
                    nc.vector.tensor_tensor(
                        sc[:, 1, lo:lo + 128], sc[:, 1, lo:lo + 128],
                        tb_sb[:], ALU.add,
                    )
                e3 = epool.tile([128, 2, 512], BF16, tag="e")
                nc.scalar.activation(e3[:, :, lo:], sc[:, :, lo:], AF.Exp)
                if kt == 0:
                    nc.vector.tensor_copy(ea_a[:], e3[:, 0])
                    nc.vector.tensor_copy(ea_b[:], e3[:, 1])
                else:
                    nc.vector.tensor_tensor(
                        ea_a[:, lo:], ea_a[:, lo:], e3[:, 0, lo:], ALU.add
                    )
                    nc.vector.tensor_tensor(
                        ea_b[:, lo:], ea_b[:, lo:], e3[:, 1, lo:], ALU.add
                    )
                nc.tensor.matmul(
                    ut_a[:, lo:],
                    v_sb[:, kt, ha * D:(ha + 1) * D],
                    e3[:, 0, lo:],
                    start=(kt == 0),
                    stop=(kt == nkt - 1),
                )
                nc.tensor.matmul(
                    ut_b[:, lo:],
                    v_sb[:, kt, hb * D:(hb + 1) * D],
                    e3[:, 1, lo:],
                    start=(kt == 0),
                    stop=(kt == nkt - 1),
                )
            for ea_h, ut_h, h in ((ea_a, ut_a, ha), (ea_b, ut_b, hb)):
                dn = dpool.tile([128, 512], F32, tag="dn")
                nc.gpsimd.partition_all_reduce(
                    dn[:], ea_h[:], 128, bass_isa.ReduceOp.add
                )
                rcp = dpool.tile([128, 512], F32, tag="rcp")
                nc.vector.reciprocal(rcp[:], dn[:])
                nc.vector.tensor_tensor(
                    oT_sb[:, h, qt * QT:(qt + 1) * QT], ut_h[:], rcp[:],
                    ALU.mult,
                )

        def out_proj(qt, tail=False):
            for qi4 in range(QT // 128):
                qi = qt * (QT // 128) + qi4
                for et in range(E // ET):
                    ps = psA.tile([128, 512], F32, tag="qkv")
                    for h in range(HPC):
                        nc.tensor.matmul(
                            ps[:],
                            oT_sb[:, h, qi * 128:(qi + 1) * 128],
                            wo_sb[:, h, et * ET:(et + 1) * ET],
                            start=(h == 0),
                            stop=(h == HPC - 1),
                        )
                    last = qi4 == QT // 128 - 1 and et == E // ET - 1
                    if tail and last:
                        for sl in range(2):
                            osb = osp.tile([128, 256], BF16, tag="osbt")
                            if sl == 0:
                                nc.vector.tensor_copy(
                                    osb[:], ps[:, 0:256])
                            else:
                                nc.scalar.copy(osb[:], ps[:, 256:512])
                            nc.sync.dma_start(
                                o[qi * 128:(qi + 1) * 128,
                                  et * ET + sl * 256:et * ET + (sl + 1) * 256],
                                osb[:],
                            )
                        continue
                    osb = osp.tile([128, 512], BF16, tag="osb")
                    if (qi * (E // ET) + et) % 2 == 0:
                        nc.vector.tensor_copy(osb[:], ps[:])
                    else:
                        nc.scalar.copy(osb[:], ps[:])
                    nc.sync.dma_start(
                        o[qi * 128:(qi + 1) * 128, et * ET:(et + 1) * ET],
                        osb[:],
                    )

        # Emission order = scheduler priority.  attn(qt) only needs
        # projection tiles 0..qt (causal), so attention interleaves into the
        # projection phase: each ACT-paced attention stretch has the next
        # projection tile's dense PE work as filler, and the late q-tiles
        # get the (deliberately delayed) out_proj groups as filler.
        tiles = [(0, 512), (512, 512), (1024, 512), (1536, 512)]
        attn_after = {0: 0, 1: 1, 2: 2, 3: 3}
        for i, (off, wdt) in enumerate(tiles):
            if i == 0:
                x_t = x_first
            elif i == 1:
                x_t = x_second
            else:
                x_t = xpool.tile([128, EO, 512], BF16, tag="x")
                nc.gpsimd.dma_start(x_t[:, :, :wdt], _xr[:, :, off:off + wdt])
            proj_tile(off, wdt, x_t)
            if i in attn_after:
                qt = attn_after[i]
                attn_pair(0, qt)
                attn_pair(1, qt)
                if qt >= 2:
                    out_proj(qt - 2)
        out_proj(NQT - 2)
        out_proj(NQT - 1)


_NC_CACHE = None


def _get_nc():
    global _NC_CACHE
    if _NC_CACHE is None:
        _NC_CACHE = build_nc()
    return _NC_CACHE


def _prep_inputs(x, Wq, bq, Wk, bk, Wv, bv, Wo, bo):
    import ml_dtypes

    BF = ml_dtypes.bfloat16
    x = np.asarray(x, dtype=np.float32)
    tb_np = np.where(
        np.arange(128)[:, None] <= np.arange(128)[None, :], 0.0, MASK_BIAS
    ).astype(np.float32)
    xT_b = [
        np.ascontiguousarray(x[b].T.astype(BF)) for b in range(B)
    ]
    Wq = np.asarray(Wq, dtype=np.float32)
    Wk = np.asarray(Wk, dtype=np.float32)
    Wv = np.asarray(Wv, dtype=np.float32)
    Wo = np.asarray(Wo, dtype=np.float32)
    bq = np.asarray(bq, dtype=np.float32)
    bk = np.asarray(bk, dtype=np.float32)
    in_maps = []
    for c in range(NCORES):
        b = c // 4
        g = c % 4
        sl = slice(g * M, (g + 1) * M)
        in_maps.append({
            "xT": xT_b[b],
            "wq": np.ascontiguousarray(Wq[sl, :].T.astype(BF)),
            "wk": np.ascontiguousarray(Wk[sl, :].T.astype(BF)),
            "wv": np.ascontiguousarray(Wv[sl, :].T.astype(BF)),
            "wo": np.ascontiguousarray(Wo[:, sl].T.astype(BF)),
            "bq": np.ascontiguousarray(
                (bq[sl] * SCALE).reshape(HPC, 128).T.astype(np.float32)
            ),
            "bk": np.ascontiguousarray(
                bk[sl].reshape(HPC, 128).T.astype(np.float32)
            ),
            "tb": tb_np,
        })
    return in_maps


def run(inputs, trace=False):
    in_maps = _prep_inputs(
        inputs["x"], inputs["Wq"], inputs["bq"], inputs["Wk"], inputs["bk"],
        inputs["Wv"], inputs["bv"], inputs["Wo"], inputs["bo"],
    )
    nc = _get_nc()
    res = run_bass_kernel_spmd(nc, in_maps, list(range(NCORES)), trace=trace)
    acc = np.zeros((B, S, E), dtype=np.float64)
    for c, r in enumerate(res.results):
        acc[c // 4] += np.asarray(r["o"]).astype(np.float64)
    acc += np.asarray(inputs["bo"], dtype=np.float64)[None, None, :]
    acc += (np.asarray(inputs["bv"], dtype=np.float64)
            @ np.asarray(inputs["Wo"], dtype=np.float64).T)[None, None, :]
    return acc.astype(np.float32), res


def kernel(**inputs):
    out, _ = run(inputs, trace=False)
    return out


# revision 62
# speedup vs baseline: 1.0008x; 1.0008x over previous
"""Multi-head causal attention on 8 TRN2 NeuronCores.

Sharding: 2-way data parallel (batch) x 4-way tensor parallel (heads).
Core c handles batch c//4 and heads (c%4)*4 .. (c%4)*4+3.  Each core
computes q/k/v projections for its 4 heads (column-sharded QKV weights),
causal attention for those heads, and the row-sharded slice of the output
projection, producing a full-shape partial output for its batch.  Host sums
the 4 partials per batch and adds bo + bv @ Wo.T (the per-head value bias
commutes through the output projection because attention rows sum to 1).

All matmuls run in bf16 (full PE rate at any moving width; fp32 PSUM
accumulate).  Layout/structure choices:
  - x is passed transposed (E-major) bf16 so QKV matmuls need no on-device
    transposes; weights are host-transposed likewise
  - scores are computed transposed [k, q] so the attn @ v matmul's operands
    arrive in exactly the layout the PE wants; heads are processed in PAIRS
    with a [128, 2, 512] PSUM score tile so one ACT exp call covers both
  - softmax denominators never touch the PE: exp tiles are accumulated over
    k-tiles with cheap 2x-mode bf16 DVE adds (per-partition partial sums,
    <=16 terms each, so bf16 rounding averages out across the 128-partition
    fp32 gpsimd partition_all_reduce), then one DVE divide normalizes
  - diagonal k-tiles only compute/accumulate their causally valid column
    sub-range (exact: diagonal tiles are last in each k-loop)
  - emission order doubles as scheduler priority: attention q-tiles
    interleave into the projection phase (causality permits it), and each
    out_proj is emitted two q-tiles late so its dependency-free matmul
    groups fill PE stalls in the ACT-paced attention inner loop; evacuation
    copies alternate DVE/ACT; PSUM budget: proj+out-proj 2 banks, paired
    scores 2x2 banks, attention accumulators 2 banks
"""

import sys

if "/opt/trn_rl_repo" not in sys.path:
    sys.path.insert(0, "/opt/trn_rl_repo")

import numpy as np

import concourse.bass as bass  # noqa: F401  (engine namespaces live on nc)
import concourse.tile as tile
from concourse import bacc, bass_isa, mybir
from concourse.bass_utils import run_bass_kernel_spmd

F32 = mybir.dt.float32
BF16 = mybir.dt.bfloat16
AF = mybir.ActivationFunctionType
ALU = mybir.AluOpType

B, S, E = 2, 2048, 2048
H, D = 16, 128
NCORES = 8
HPC = 4                    # heads per core
M = HPC * D                # local qkv channels per core = 512
EO = E // 128              # 16 contraction chunks
XT = 512                   # token-tile width for projections
NT = S // XT               # 4 token tiles
QT = 512                   # q-tile width for attention
NQT = S // QT              # 4 q-tiles
ET = 512                   # e-tile width for out-projection
SCALE = 1.0 / float(np.sqrt(D))
MASK_BIAS = -30.0


def build_nc():
    nc = bacc.Bacc(trn_type="TRN2", target_bir_lowering=False, num_swdge_queues=4)

    xT = nc.declare_dram_parameter("xT", [E, S], BF16, isOutput=False)
    wq = nc.declare_dram_parameter("wq", [E, M], BF16, isOutput=False)
    wk = nc.declare_dram_parameter("wk", [E, M], BF16, isOutput=False)
    wv = nc.declare_dram_parameter("wv", [E, M], BF16, isOutput=False)
    wo = nc.declare_dram_parameter("wo", [M, E], BF16, isOutput=False)
    bq = nc.declare_dram_parameter("bq", [128, HPC], F32, isOutput=False)
    bk = nc.declare_dram_parameter("bk", [128, HPC], F32, isOutput=False)
    tb = nc.declare_dram_parameter("tb", [128, 128], F32, isOutput=False)
    o = nc.declare_dram_parameter("o", [S, E], BF16, isOutput=True)

    with tile.TileContext(nc) as tc:
        _body(tc, nc, xT, wq, wk, wv, wo, bq, bk, tb, o)
    nc.compile()
    return nc


def _body(tc, nc, xT, wq, wk, wv, wo, bq, bk, tb, o):
    from contextlib import ExitStack

    ctx = ExitStack()
    with ctx:
        # PSUM pools first so the paired score tile lands bank-aligned.
        psS = ctx.enter_context(tc.tile_pool(name="psS", bufs=2, space="PSUM"))
        psA = ctx.enter_context(tc.tile_pool(name="psA", bufs=2, space="PSUM"))
        psU = ctx.enter_context(tc.tile_pool(name="psU", bufs=2, space="PSUM"))
        wpool = ctx.enter_context(tc.tile_pool(name="w", bufs=1))
        xpool = ctx.enter_context(tc.tile_pool(name="x", bufs=2))
        qkv = ctx.enter_context(tc.tile_pool(name="qkv", bufs=1))
        otp = ctx.enter_context(tc.tile_pool(name="ot", bufs=1))
        epool = ctx.enter_context(tc.tile_pool(name="e", bufs=6))
        eap = ctx.enter_context(tc.tile_pool(name="ea", bufs=4))
        dpool = ctx.enter_context(tc.tile_pool(name="dn", bufs=3))
        osp = ctx.enter_context(tc.tile_pool(name="os", bufs=8))

        # ---- weights / constants ----
        wq_sb = wpool.tile([128, EO, M], BF16, tag="wq")
        wk_sb = wpool.tile([128, EO, M], BF16, tag="wk")
        wv_sb = wpool.tile([128, EO, M], BF16, tag="wv")
        wo_sb = wpool.tile([128, HPC, E], BF16, tag="wo")

        # Warm the PE (HAM clock gate) with tiny matmuls while x0/weights
        # stream in; the warm tiles come from memsets so they have no DMA
        # dependency.
        warm_l = wpool.tile([128, 1], BF16, tag="wl")
        warm_r = wpool.tile([128, 512], BF16, tag="wr")
        nc.vector.memset(warm_l[:], 0.0)
        nc.vector.memset(warm_r[:], 0.0)
        warm = psS.tile([128, 2, 512], F32, tag="sc")
        for _ in range(18):
            nc.tensor.matmul(warm[:1, 0, :], warm_l[:], warm_r[:],
                             start=True, stop=True)

        # x tile 0 and the first head-pair's wq/wk stream in first so the
        # first projection groups start as soon as possible; wv is needed by
        # the v-projection at the end of tile 0, wo only at the first
        # out-projection.
        _wqr = wq.rearrange("(eo p) m -> p eo m", p=128)
        _wkr = wk.rearrange("(eo p) m -> p eo m", p=128)
        _xr = xT.rearrange("(eo p) s -> p eo s", p=128)
        x_first = xpool.tile([128, EO, 512], BF16, tag="x")
        nc.sync.dma_start(x_first[:, 0:8], _xr[:, 0:8, 0:512])
        nc.sync.dma_start(wq_sb[:, :, 0:256], _wqr[:, :, 0:256])
        nc.gpsimd.dma_start(x_first[:, 8:16], _xr[:, 8:16, 0:512])
        nc.gpsimd.dma_start(wk_sb[:, :, 0:256], _wkr[:, :, 0:256])
        nc.gpsimd.dma_start(wq_sb[:, :, 256:512], _wqr[:, :, 256:512])
        nc.gpsimd.dma_start(wk_sb[:, :, 256:512], _wkr[:, :, 256:512])
        nc.gpsimd.dma_start(wv_sb[:], wv.rearrange("(eo p) m -> p eo m", p=128))
        x_second = xpool.tile([128, EO, 512], BF16, tag="x")
        nc.gpsimd.dma_start(x_second[:], _xr[:, :, 512:1024])
        nc.gpsimd.dma_start(wo_sb[:], wo.rearrange("(h p) e -> p h e", p=128))

        bq_sb = wpool.tile([128, HPC], F32, tag="bq")
        bk_sb = wpool.tile([128, HPC], F32, tag="bk")
        tb_sb = wpool.tile([128, 128], F32, tag="tb")
        nc.sync.dma_start(bq_sb[:], bq[:])
        nc.sync.dma_start(bk_sb[:], bk[:])
        nc.sync.dma_start(tb_sb[:], tb[:])

        qT_sb = qkv.tile([128, HPC, S], BF16, tag="qT")
        kT_sb = qkv.tile([128, HPC, S], BF16, tag="kT")
        v_sb = qkv.tile([128, S // 128, M], BF16, tag="v")
        oT_sb = otp.tile([128, HPC, S], BF16, tag="oT")

        def proj_group(off, wdt, x_t, h, w_sb, dst, bias, scl):
            ps = psA.tile([128, 512], F32, tag="qkv")
            for eo in range(EO):
                nc.tensor.matmul(
                    ps[:, :wdt],
                    w_sb[:, eo, h * D:(h + 1) * D],
                    x_t[:, eo, :wdt],
                    start=(eo == 0),
                    stop=(eo == EO - 1),
                )
            nc.scalar.activation(
                dst[:, h, off:off + wdt],
                ps[:, :wdt],
                AF.Identity,
                bias=bias[:, h:h + 1],
                scale=scl,
            )

        def proj_tile(off, wdt, x_t):
            # head-pair-major order so the first groups only need the first
            # half of wq/wk
            for hp in (0, 1):
                for h in (2 * hp, 2 * hp + 1):
                    proj_group(off, wdt, x_t, h, wq_sb, qT_sb, bq_sb, SCALE)
                for h in (2 * hp, 2 * hp + 1):
                    proj_group(off, wdt, x_t, h, wk_sb, kT_sb, bk_sb, 1.0)
            for st in range(wdt // 128):
                ps = psA.tile([128, 512], F32, tag="qkv")
                for eo in range(EO):
                    nc.tensor.matmul(
                        ps[:],
                        x_t[:, eo, st * 128:(st + 1) * 128],
                        wv_sb[:, eo, :],
                        start=(eo == 0),
                        stop=(eo == EO - 1),
                    )
                nc.vector.tensor_copy(v_sb[:, off // 128 + st, :], ps[:])

        def attn_pair(hp, qt):
            ha, hb = 2 * hp, 2 * hp + 1
            q_a = qT_sb[:, ha, qt * QT:(qt + 1) * QT]
            q_b = qT_sb[:, hb, qt * QT:(qt + 1) * QT]
            ut_a = psU.tile([128, 512], F32, tag="ut")
            ut_b = psU.tile([128, 512], F32, tag="ut")
            ea_a = eap.tile([128, 512], BF16, tag="ea")
            ea_b = eap.tile([128, 512], BF16, tag="ea")
            nkt = (qt + 1) * (QT // 128)
            for kt in range(nkt):
                jj = kt - qt * (QT // 128)
                # columns < jj*128 of this k-tile's block are causally
                # masked; diagonal tiles come last in the k-loop, so
                # accumulating only the valid sub-range is exact.
                lo = max(jj, 0) * 128
                sc = psS.tile([128, 2, 512], F32, tag="sc")
                for j, q_h, hh in ((0, q_a, ha), (1, q_b, hb)):
                    nc.tensor.matmul(
                        sc[:, j, lo:],
                        kT_sb[:, hh, kt * 128:(kt + 1) * 128],
                        q_h[:, lo:],
                        start=True,
                        stop=True,
                    )
                if jj >= 0:
                    nc.vector.tensor_tensor(
                        sc[:, 0, lo:lo + 128], sc[:, 0, lo:lo + 128],
                        tb_sb[:], ALU.add,
                    )
                    nc.vector.tensor_tensor(
                        sc[:, 1, lo:lo + 128], sc[:, 1, lo:lo + 128],
                        tb_sb[:], ALU.add,
                    )
                e3 = epool.tile([128, 2, 512], BF16, tag="e")
                nc.scalar.activation(e3[:, :, lo:], sc[:, :, lo:], AF.Exp)
                if kt == 0:
                    nc.vector.tensor_copy(ea_a[:], e3[:, 0])
                    nc.vector.tensor_copy(ea_b[:], e3[:, 1])
                else:
                    nc.vector.tensor_tensor(
                        ea_a[:, lo:], ea_a[:, lo:], e3[:, 0, lo:], ALU.add
                    )
                    nc.vector.tensor_tensor(
                        ea_b[:, lo:], ea_b[:, lo:], e3[:, 1, lo:], ALU.add
                    )
                nc.tensor.matmul(
                    ut_a[:, lo:],
                    v_sb[:, kt, ha * D:(ha + 1) * D],
                    e3[:, 0, lo:],
                    start=(kt == 0),
                    stop=(kt == nkt - 1),
                )
                nc.tensor.matmul(
                    ut_b[:, lo:],
                    v_sb[:, kt, hb * D:(hb + 1) * D],
                    e3[:, 1, lo:],
                    start=(kt == 0),
                    stop=(kt == nkt - 1),
                )
            for ea_h, ut_h, h in ((ea_a, ut_a, ha), (ea_b, ut_b, hb)):
                dn = dpool.tile([128, 512], F32, tag="dn")
                nc.gpsimd.partition_all_reduce(
                    dn[:], ea_h[:], 128, bass_isa.ReduceOp.add
                )
                rcp = dpool.tile([128, 512], F32, tag="rcp")
                nc.vector.reciprocal(rcp[:], dn[:])
                nc.vector.tensor_tensor(
                    oT_sb[:, h, qt * QT:(qt + 1) * QT], ut_h[:], rcp[:],
                    ALU.mult,
                )

        def out_proj(qt, tail=False, half=None):
            qrange = range(QT // 128)
            if half == 0:
                qrange = range(0, QT // 256)
            elif half == 1:
                qrange = range(QT // 256, QT // 128)
            for qi4 in qrange:
                qi = qt * (QT // 128) + qi4
                for et in range(E // ET):
                    ps = psA.tile([128, 512], F32, tag="qkv")
                    for h in range(HPC):
                        nc.tensor.matmul(
                            ps[:],
                            oT_sb[:, h, qi * 128:(qi + 1) * 128],
                            wo_sb[:, h, et * ET:(et + 1) * ET],
                            start=(h == 0),
                            stop=(h == HPC - 1),
                        )
                    last = qi4 == QT // 128 - 1 and et == E // ET - 1
                    if tail and last:
                        for sl in range(2):
                            osb = osp.tile([128, 256], BF16, tag="osbt")
                            if sl == 0:
                                nc.vector.tensor_copy(
                                    osb[:], ps[:, 0:256])
                            else:
                                nc.scalar.copy(osb[:], ps[:, 256:512])
                            nc.sync.dma_start(
                                o[qi * 128:(qi + 1) * 128,
                                  et * ET + sl * 256:et * ET + (sl + 1) * 256],
                                osb[:],
                            )
                        continue
                    osb = osp.tile([128, 512], BF16, tag="osb")
                    if (qi * (E // ET) + et) % 2 == 0:
                        nc.vector.tensor_copy(osb[:], ps[:])
                    else:
                        nc.scalar.copy(osb[:], ps[:])
                    nc.sync.dma_start(
                        o[qi * 128:(qi + 1) * 128, et * ET:(et + 1) * ET],
                        osb[:],
                    )

        # Emission order = scheduler priority.  attn(qt) only needs
        # projection tiles 0..qt (causal), so attention interleaves into the
        # projection phase: each ACT-paced attention stretch has the next
        # projection tile's dense PE work as filler, and the late q-tiles
        # get the (deliberately delayed) out_proj groups as filler.
        tiles = [(0, 512), (512, 512), (1024, 512), (1536, 512)]
        attn_after = {0: 0, 1: 1, 2: 2, 3: 3}
        for i, (off, wdt) in enumerate(tiles):
            if i == 0:
                x_t = x_first
            elif i == 1:
                x_t = x_second
            else:
                x_t = xpool.tile([128, EO, 512], BF16, tag="x")
                nc.gpsimd.dma_start(x_t[:, :, :wdt], _xr[:, :, off:off + wdt])
            proj_tile(off, wdt, x_t)
            if i in attn_after:
                qt = attn_after[i]
                attn_pair(0, qt)
                if qt >= 2:
                    out_proj(qt - 2, half=0)
                attn_pair(1, qt)
                if qt >= 2:
                    out_proj(qt - 2, half=1)
                if qt == NQT - 1:
                    out_proj(NQT - 2)
        out_proj(NQT - 1)


_NC_CACHE = None


def _get_nc():
    global _NC_CACHE
    if _NC_CACHE is None:
        _NC_CACHE = build_nc()
    return _NC_CACHE


def _prep_inputs(x, Wq, bq, Wk, bk, Wv, bv, Wo, bo):
    import ml_dtypes

    BF = ml_dtypes.bfloat16
    x = np.asarray(x, dtype=np.float32)
    tb_np = np.where(
        np.arange(128)[:, None] <= np.arange(128)[None, :], 0.0, MASK_BIAS
    ).astype(np.float32)
    xT_b = [
        np.ascontiguousarray(x[b].T.astype(BF)) for b in range(B)
    ]
    Wq = np.asarray(Wq, dtype=np.float32)
    Wk = np.asarray(Wk, dtype=np.float32)
    Wv = np.asarray(Wv, dtype=np.float32)
    Wo = np.asarray(Wo, dtype=np.float32)
    bq = np.asarray(bq, dtype=np.float32)
    bk = np.asarray(bk, dtype=np.float32)
    in_maps = []
    for c in range(NCORES):
        b = c // 4
        g = c % 4
        sl = slice(g * M, (g + 1) * M)
        in_maps.append({
            "xT": xT_b[b],
            "wq": np.ascontiguousarray(Wq[sl, :].T.astype(BF)),
            "wk": np.ascontiguousarray(Wk[sl, :].T.astype(BF)),
            "wv": np.ascontiguousarray(Wv[sl, :].T.astype(BF)),
            "wo": np.ascontiguousarray(Wo[:, sl].T.astype(BF)),
            "bq": np.ascontiguousarray(
                (bq[sl] * SCALE).reshape(HPC, 128).T.astype(np.float32)
            ),
            "bk": np.ascontiguousarray(
                bk[sl].reshape(HPC, 128).T.astype(np.float32)
            ),
            "tb": tb_np,
        })
    return in_maps


def run(inputs, trace=False):
    in_maps = _prep_inputs(
        inputs["x"], inputs["Wq"], inputs["bq"], inputs["Wk"], inputs["bk"],
        inputs["Wv"], inputs["bv"], inputs["Wo"], inputs["bo"],
    )
    nc = _get_nc()
    res = run_bass_kernel_spmd(nc, in_maps, list(range(NCORES)), trace=trace)
    acc = np.zeros((B, S, E), dtype=np.float64)
    for c, r in enumerate(res.results):
        acc[c // 4] += np.asarray(r["o"]).astype(np.float64)
    acc += np.asarray(inputs["bo"], dtype=np.float64)[None, None, :]
    acc += (np.asarray(inputs["bv"], dtype=np.float64)
            @ np.asarray(inputs["Wo"], dtype=np.float64).T)[None, None, :]
    return acc.astype(np.float32), res


def kernel(**inputs):
    out, _ = run(inputs, trace=False)
    return out


# revision 63
# speedup vs baseline: 1.2631x; 1.2621x over previous
"""Multi-head causal attention on 8 TRN2 NeuronCores.

Sharding: 2-way data parallel (batch) x 4-way tensor parallel (heads).
Core c handles batch c//4 and heads (c%4)*4 .. (c%4)*4+3.  Each core
computes q/k/v projections for its 4 heads (column-sharded QKV weights),
causal attention for those heads, and the row-sharded slice of the output
projection, producing a full-shape partial output for its batch.  Host sums
the 4 partials per batch and adds bo + bv @ Wo.T (the per-head value bias
commutes through the output projection because attention rows sum to 1).

All matmuls run in bf16 (full PE rate at any moving width; fp32 PSUM
accumulate).  Layout/structure choices:
  - x is passed transposed (E-major) bf16 so QKV matmuls need no on-device
    transposes; weights are host-transposed likewise
  - scores are computed transposed [k, q] so the attn @ v matmul's operands
    arrive in exactly the layout the PE wants; heads are processed in PAIRS
    with a [128, 2, 512] PSUM score tile so one ACT exp call covers both
  - softmax denominators never touch the PE: exp tiles are accumulated over
    k-tiles with cheap 2x-mode bf16 DVE adds (per-partition partial sums,
    <=16 terms each, so bf16 rounding averages out across the 128-partition
    fp32 gpsimd partition_all_reduce), then one DVE divide normalizes
  - diagonal k-tiles only compute/accumulate their causally valid column
    sub-range (exact: diagonal tiles are last in each k-loop)
  - emission order doubles as scheduler priority: attention q-tiles
    interleave into the projection phase (causality permits it), and each
    out_proj is emitted two q-tiles late so its dependency-free matmul
    groups fill PE stalls in the ACT-paced attention inner loop; evacuation
    copies alternate DVE/ACT; PSUM budget: proj+out-proj 2 banks, paired
    scores 2x2 banks, attention accumulators 2 banks
"""

import sys

if "/opt/trn_rl_repo" not in sys.path:
    sys.path.insert(0, "/opt/trn_rl_repo")

import numpy as np

import concourse.bass as bass  # noqa: F401  (engine namespaces live on nc)
import concourse.tile as tile
from concourse import bacc, bass_isa, mybir
from concourse.bass_utils import run_bass_kernel_spmd

F32 = mybir.dt.float32
FP8 = mybir.dt.float8e4
BF16 = mybir.dt.bfloat16
AF = mybir.ActivationFunctionType
ALU = mybir.AluOpType

B, S, E = 2, 2048, 2048
H, D = 16, 128
NCORES = 8
HPC = 4                    # heads per core
M = HPC * D                # local qkv channels per core = 512
EO = E // 128              # 16 contraction chunks
XT = 512                   # token-tile width for projections
NT = S // XT               # 4 token tiles
QT = 512                   # q-tile width for attention
NQT = S // QT              # 4 q-tiles
ET = 512                   # e-tile width for out-projection
SCALE = 1.0 / float(np.sqrt(D))
MASK_BIAS = -30.0


def build_nc():
    nc = bacc.Bacc(trn_type="TRN2", target_bir_lowering=False, num_swdge_queues=4)

    xT = nc.declare_dram_parameter("xT", [E, S], BF16, isOutput=False)
    x8 = nc.declare_dram_parameter("x8", [E, S], FP8, isOutput=False)
    wq = nc.declare_dram_parameter("wq", [E, M], FP8, isOutput=False)
    wk = nc.declare_dram_parameter("wk", [E, M], FP8, isOutput=False)
    wv = nc.declare_dram_parameter("wv", [E, M], BF16, isOutput=False)
    wo = nc.declare_dram_parameter("wo", [M, E], BF16, isOutput=False)
    bq = nc.declare_dram_parameter("bq", [128, HPC], F32, isOutput=False)
    bk = nc.declare_dram_parameter("bk", [128, HPC], F32, isOutput=False)
    tb = nc.declare_dram_parameter("tb", [128, 128], F32, isOutput=False)
    o = nc.declare_dram_parameter("o", [S, E], BF16, isOutput=True)

    with tile.TileContext(nc) as tc:
        _body(tc, nc, xT, x8, wq, wk, wv, wo, bq, bk, tb, o)
    nc.compile()
    return nc


def _body(tc, nc, xT, x8, wq, wk, wv, wo, bq, bk, tb, o):
    from contextlib import ExitStack

    ctx = ExitStack()
    with ctx:
        # PSUM pools first so the paired score tile lands bank-aligned.
        psS = ctx.enter_context(tc.tile_pool(name="psS", bufs=2, space="PSUM"))
        psA = ctx.enter_context(tc.tile_pool(name="psA", bufs=2, space="PSUM"))
        psU = ctx.enter_context(tc.tile_pool(name="psU", bufs=2, space="PSUM"))
        wpool = ctx.enter_context(tc.tile_pool(name="w", bufs=1))
        xpool = ctx.enter_context(tc.tile_pool(name="x", bufs=2))
        x8pool = ctx.enter_context(tc.tile_pool(name="x8", bufs=2))
        qkv = ctx.enter_context(tc.tile_pool(name="qkv", bufs=1))
        otp = ctx.enter_context(tc.tile_pool(name="ot", bufs=1))
        epool = ctx.enter_context(tc.tile_pool(name="e", bufs=6))
        eap = ctx.enter_context(tc.tile_pool(name="ea", bufs=4))
        dpool = ctx.enter_context(tc.tile_pool(name="dn", bufs=3))
        osp = ctx.enter_context(tc.tile_pool(name="os", bufs=8))

        # ---- weights / constants ----
        wq_sb = wpool.tile([128, EO, M], FP8, tag="wq")
        wk_sb = wpool.tile([128, EO, M], FP8, tag="wk")
        wv_sb = wpool.tile([128, EO, M], BF16, tag="wv")
        wo_sb = wpool.tile([128, HPC, E], BF16, tag="wo")

        # Warm the PE (HAM clock gate) with tiny matmuls while x0/weights
        # stream in; the warm tiles come from memsets so they have no DMA
        # dependency.
        warm_l = wpool.tile([128, 1], BF16, tag="wl")
        warm_r = wpool.tile([128, 512], BF16, tag="wr")
        nc.vector.memset(warm_l[:], 0.0)
        nc.vector.memset(warm_r[:], 0.0)
        warm = psS.tile([128, 2, 512], F32, tag="sc")
        for _ in range(18):
            nc.tensor.matmul(warm[:1, 0, :], warm_l[:], warm_r[:],
                             start=True, stop=True)

        # x tile 0 and the first head-pair's wq/wk stream in first so the
        # first projection groups start as soon as possible; wv is needed by
        # the v-projection at the end of tile 0, wo only at the first
        # out-projection.
        _wqr = wq.rearrange("(eo p) m -> p eo m", p=128)
        _wkr = wk.rearrange("(eo p) m -> p eo m", p=128)
        _xr = xT.rearrange("(eo p) s -> p eo s", p=128)
        _x8r = x8.rearrange("(eo p) s -> p eo s", p=128)
        x8_first = x8pool.tile([128, EO, 512], FP8, tag="x8")
        x_first = xpool.tile([128, EO, 512], BF16, tag="x")
        nc.sync.dma_start(x8_first[:], _x8r[:, :, 0:512])
        nc.sync.dma_start(wq_sb[:], _wqr[:])
        nc.gpsimd.dma_start(wk_sb[:], _wkr[:])
        nc.gpsimd.dma_start(x_first[:], _xr[:, :, 0:512])
        nc.gpsimd.dma_start(wv_sb[:], wv.rearrange("(eo p) m -> p eo m", p=128))
        x8_second = x8pool.tile([128, EO, 512], FP8, tag="x8")
        nc.gpsimd.dma_start(x8_second[:], _x8r[:, :, 512:1024])
        x_second = xpool.tile([128, EO, 512], BF16, tag="x")
        nc.gpsimd.dma_start(x_second[:], _xr[:, :, 512:1024])
        nc.gpsimd.dma_start(wo_sb[:], wo.rearrange("(h p) e -> p h e", p=128))

        bq_sb = wpool.tile([128, HPC], F32, tag="bq")
        bk_sb = wpool.tile([128, HPC], F32, tag="bk")
        tb_sb = wpool.tile([128, 128], F32, tag="tb")
        nc.sync.dma_start(bq_sb[:], bq[:])
        nc.sync.dma_start(bk_sb[:], bk[:])
        nc.sync.dma_start(tb_sb[:], tb[:])

        qT_sb = qkv.tile([128, HPC, S], BF16, tag="qT")
        kT_sb = qkv.tile([128, HPC, S], BF16, tag="kT")
        v_sb = qkv.tile([128, S // 128, M], BF16, tag="v")
        oT_sb = otp.tile([128, HPC, S], BF16, tag="oT")

        def proj_group(off, wdt, x8_t, h, w_sb, dst, bias, scl):
            # fp8 DoubleRow: each matmul contracts 256 channels (2 eo-pairs
            # packed on the partition dim); weights are host-prescaled by 64
            # and the evac scale divides it back out
            ps = psA.tile([128, 512], F32, tag="qkv")
            for j in range(EO // 2):
                nc.tensor.matmul(
                    ps[:, :wdt],
                    w_sb[:, 2 * j:2 * j + 2, h * D:(h + 1) * D],
                    x8_t[:, 2 * j:2 * j + 2, :wdt],
                    start=(j == 0),
                    stop=(j == EO // 2 - 1),
                    perf_mode=mybir.MatmulPerfMode.DoubleRow,
                )
            nc.scalar.activation(
                dst[:, h, off:off + wdt],
                ps[:, :wdt],
                AF.Identity,
                bias=bias[:, h:h + 1],
                scale=scl,
            )

        def proj_tile(off, wdt, x_t, x8_t):
            # head-pair-major order so the first groups only need the first
            # half of wq/wk
            for hp in (0, 1):
                for h in (2 * hp, 2 * hp + 1):
                    proj_group(off, wdt, x8_t, h, wq_sb, qT_sb, bq_sb, SCALE / 64.0)
                for h in (2 * hp, 2 * hp + 1):
                    proj_group(off, wdt, x8_t, h, wk_sb, kT_sb, bk_sb, 1.0 / 64.0)
            for st in range(wdt // 128):
                ps = psA.tile([128, 512], F32, tag="qkv")
                for eo in range(EO):
                    nc.tensor.matmul(
                        ps[:],
                        x_t[:, eo, st * 128:(st + 1) * 128],
                        wv_sb[:, eo, :],
                        start=(eo == 0),
                        stop=(eo == EO - 1),
                    )
                nc.vector.tensor_copy(v_sb[:, off // 128 + st, :], ps[:])

        def attn_pair(hp, qt):
            ha, hb = 2 * hp, 2 * hp + 1
            q_a = qT_sb[:, ha, qt * QT:(qt + 1) * QT]
            q_b = qT_sb[:, hb, qt * QT:(qt + 1) * QT]
            ut_a = psU.tile([128, 512], F32, tag="ut")
            ut_b = psU.tile([128, 512], F32, tag="ut")
            ea_a = eap.tile([128, 512], BF16, tag="ea")
            ea_b = eap.tile([128, 512], BF16, tag="ea")
            nkt = (qt + 1) * (QT // 128)
            for kt in range(nkt):
                jj = kt - qt * (QT // 128)
                # columns < jj*128 of this k-tile's block are causally
                # masked; diagonal tiles come last in the k-loop, so
                # accumulating only the valid sub-range is exact.
                lo = max(jj, 0) * 128
                sc = psS.tile([128, 2, 512], F32, tag="sc")
                for j, q_h, hh in ((0, q_a, ha), (1, q_b, hb)):
                    nc.tensor.matmul(
                        sc[:, j, lo:],
                        kT_sb[:, hh, kt * 128:(kt + 1) * 128],
                        q_h[:, lo:],
                        start=True,
                        stop=True,
                    )
                if jj >= 0:
                    nc.vector.tensor_tensor(
                        sc[:, 0, lo:lo + 128], sc[:, 0, lo:lo + 128],
                        tb_sb[:], ALU.add,
                    )
                    nc.vector.tensor_tensor(
                        sc[:, 1, lo:lo + 128], sc[:, 1, lo:lo + 128],
                        tb_sb[:], ALU.add,
                    )
                e3 = epool.tile([128, 2, 512], BF16, tag="e")
                nc.scalar.activation(e3[:, :, lo:], sc[:, :, lo:], AF.Exp)
                if kt == 0:
                    nc.vector.tensor_copy(ea_a[:], e3[:, 0])
                    nc.vector.tensor_copy(ea_b[:], e3[:, 1])
                else:
                    nc.vector.tensor_tensor(
                        ea_a[:, lo:], ea_a[:, lo:], e3[:, 0, lo:], ALU.add
                    )
                    nc.vector.tensor_tensor(
                        ea_b[:, lo:], ea_b[:, lo:], e3[:, 1, lo:], ALU.add
                    )
                nc.tensor.matmul(
                    ut_a[:, lo:],
                    v_sb[:, kt, ha * D:(ha + 1) * D],
                    e3[:, 0, lo:],
                    start=(kt == 0),
                    stop=(kt == nkt - 1),
                )
                nc.tensor.matmul(
                    ut_b[:, lo:],
                    v_sb[:, kt, hb * D:(hb + 1) * D],
                    e3[:, 1, lo:],
                    start=(kt == 0),
                    stop=(kt == nkt - 1),
                )
            for ea_h, ut_h, h in ((ea_a, ut_a, ha), (ea_b, ut_b, hb)):
                dn = dpool.tile([128, 512], F32, tag="dn")
                nc.gpsimd.partition_all_reduce(
                    dn[:], ea_h[:], 128, bass_isa.ReduceOp.add
                )
                rcp = dpool.tile([128, 512], F32, tag="rcp")
                nc.vector.reciprocal(rcp[:], dn[:])
                nc.vector.tensor_tensor(
                    oT_sb[:, h, qt * QT:(qt + 1) * QT], ut_h[:], rcp[:],
                    ALU.mult,
                )

        def out_proj(qt, tail=False, half=None):
            qrange = range(QT // 128)
            if half == 0:
                qrange = range(0, QT // 256)
            elif half == 1:
                qrange = range(QT // 256, QT // 128)
            for qi4 in qrange:
                qi = qt * (QT // 128) + qi4
                for et in range(E // ET):
                    ps = psA.tile([128, 512], F32, tag="qkv")
                    for h in range(HPC):
                        nc.tensor.matmul(
                            ps[:],
                            oT_sb[:, h, qi * 128:(qi + 1) * 128],
                            wo_sb[:, h, et * ET:(et + 1) * ET],
                            start=(h == 0),
                            stop=(h == HPC - 1),
                        )
                    last = qi4 == QT // 128 - 1 and et == E // ET - 1
                    if tail and last:
                        for sl in range(2):
                            osb = osp.tile([128, 256], BF16, tag="osbt")
                            if sl == 0:
                                nc.vector.tensor_copy(
                                    osb[:], ps[:, 0:256])
                            else:
                                nc.scalar.copy(osb[:], ps[:, 256:512])
                            nc.sync.dma_start(
                                o[qi * 128:(qi + 1) * 128,
                                  et * ET + sl * 256:et * ET + (sl + 1) * 256],
                                osb[:],
                            )
                        continue
                    osb = osp.tile([128, 512], BF16, tag="osb")
                    if (qi * (E // ET) + et) % 2 == 0:
                        nc.vector.tensor_copy(osb[:], ps[:])
                    else:
                        nc.scalar.copy(osb[:], ps[:])
                    nc.sync.dma_start(
                        o[qi * 128:(qi + 1) * 128, et * ET:(et + 1) * ET],
                        osb[:],
                    )

        # Emission order = scheduler priority.  attn(qt) only needs
        # projection tiles 0..qt (causal), so attention interleaves into the
        # projection phase: each ACT-paced attention stretch has the next
        # projection tile's dense PE work as filler, and the late q-tiles
        # get the (deliberately delayed) out_proj groups as filler.
        tiles = [(0, 512), (512, 512), (1024, 512), (1536, 512)]
        attn_after = {0: 0, 1: 1, 2: 2, 3: 3}
        for i, (off, wdt) in enumerate(tiles):
            if i == 0:
                x_t, x8_t = x_first, x8_first
            elif i == 1:
                x_t, x8_t = x_second, x8_second
            else:
                x8_t = x8pool.tile([128, EO, 512], FP8, tag="x8")
                nc.gpsimd.dma_start(x8_t[:, :, :wdt], _x8r[:, :, off:off + wdt])
                x_t = xpool.tile([128, EO, 512], BF16, tag="x")
                nc.gpsimd.dma_start(x_t[:, :, :wdt], _xr[:, :, off:off + wdt])
            proj_tile(off, wdt, x_t, x8_t)
            if i in attn_after:
                qt = attn_after[i]
                attn_pair(0, qt)
                if qt >= 2:
                    out_proj(qt - 2, half=0)
                attn_pair(1, qt)
                if qt >= 2:
                    out_proj(qt - 2, half=1)
                if qt == NQT - 1:
                    out_proj(NQT - 2)
        out_proj(NQT - 1)


_NC_CACHE = None


def _get_nc():
    global _NC_CACHE
    if _NC_CACHE is None:
        _NC_CACHE = build_nc()
    return _NC_CACHE


def _prep_inputs(x, Wq, bq, Wk, bk, Wv, bv, Wo, bo):
    import ml_dtypes

    BF = ml_dtypes.bfloat16
    x = np.asarray(x, dtype=np.float32)
    tb_np = np.where(
        np.arange(128)[:, None] <= np.arange(128)[None, :], 0.0, MASK_BIAS
    ).astype(np.float32)
    F8 = ml_dtypes.float8_e4m3fn
    xT_b = [
        np.ascontiguousarray(x[b].T.astype(BF)) for b in range(B)
    ]
    x8_b = [
        np.ascontiguousarray(x[b].T.astype(F8)) for b in range(B)
    ]
    Wq = np.asarray(Wq, dtype=np.float32)
    Wk = np.asarray(Wk, dtype=np.float32)
    Wv = np.asarray(Wv, dtype=np.float32)
    Wo = np.asarray(Wo, dtype=np.float32)
    bq = np.asarray(bq, dtype=np.float32)
    bk = np.asarray(bk, dtype=np.float32)
    in_maps = []
    for c in range(NCORES):
        b = c // 4
        g = c % 4
        sl = slice(g * M, (g + 1) * M)
        in_maps.append({
            "xT": xT_b[b],
            "x8": x8_b[b],
            "wq": np.ascontiguousarray((Wq[sl, :].T * 64.0).astype(F8)),
            "wk": np.ascontiguousarray((Wk[sl, :].T * 64.0).astype(F8)),
            "wv": np.ascontiguousarray(Wv[sl, :].T.astype(BF)),
            "wo": np.ascontiguousarray(Wo[:, sl].T.astype(BF)),
            "bq": np.ascontiguousarray(
                (bq[sl] * SCALE).reshape(HPC, 128).T.astype(np.float32)
            ),
            "bk": np.ascontiguousarray(
                bk[sl].reshape(HPC, 128).T.astype(np.float32)
            ),
            "tb": tb_np,
        })
    return in_maps


def run(inputs, trace=False):
    in_maps = _prep_inputs(
        inputs["x"], inputs["Wq"], inputs["bq"], inputs["Wk"], inputs["bk"],
        inputs["Wv"], inputs["bv"], inputs["Wo"], inputs["bo"],
    )
    nc = _get_nc()
    res = run_bass_kernel_spmd(nc, in_maps, list(range(NCORES)), trace=trace)
    acc = np.zeros((B, S, E), dtype=np.float64)
    for c, r in enumerate(res.results):
        acc[c // 4] += np.asarray(r["o"]).astype(np.float64)
    acc += np.asarray(inputs["bo"], dtype=np.float64)[None, None, :]
    acc += (np.asarray(inputs["bv"], dtype=np.float64)
            @ np.asarray(inputs["Wo"], dtype=np.float64).T)[None, None, :]
    return acc.astype(np.float32), res


def kernel(**inputs):
    out, _ = run(inputs, trace=False)
    return out


# revision 73
# speedup vs baseline: 1.2913x; 1.0223x over previous
"""Multi-head causal attention on 8 TRN2 NeuronCores.

Sharding: 2-way data parallel (batch) x 4-way tensor parallel (heads).
Core c handles batch c//4 and heads (c%4)*4 .. (c%4)*4+3.  Each core
computes q/k/v projections for its 4 heads (column-sharded QKV weights),
causal attention for those heads, and the row-sharded slice of the output
projection, producing a full-shape partial output for its batch.  Host sums
the 4 partials per batch and adds bo + bv @ Wo.T (the per-head value bias
commutes through the output projection because attention rows sum to 1).

Matmuls run in bf16 (full PE rate, fp32 PSUM accumulate), except the q/k
projections which use fp8e4m3 DoubleRow (2 contraction rows per PE cell;
weights host-prescaled by 64, divided back out in the evacuation scale).
fp8 quantization noise in q/k washes out through the softmax; v and the
output path stay bf16 because early causal rows expose v errors directly.
Layout/structure choices:
  - x is passed transposed (E-major) bf16 so QKV matmuls need no on-device
    transposes; weights are host-transposed likewise
  - scores are computed transposed [k, q] so the attn @ v matmul's operands
    arrive in exactly the layout the PE wants; heads are processed in PAIRS
    with a [128, 2, 512] PSUM score tile so one ACT exp call covers both
  - softmax denominators never touch the PE: exp tiles are accumulated over
    k-tiles with cheap 2x-mode bf16 DVE adds (per-partition partial sums,
    <=16 terms each, so bf16 rounding averages out across the 128-partition
    fp32 gpsimd partition_all_reduce), then one DVE divide normalizes
  - diagonal k-tiles only compute/accumulate their causally valid column
    sub-range (exact: diagonal tiles are last in each k-loop)
  - emission order doubles as scheduler priority: attention q-tiles
    interleave into the projection phase (causality permits it), and each
    out_proj is emitted two q-tiles late so its dependency-free matmul
    groups fill PE stalls in the ACT-paced attention inner loop; evacuation
    copies alternate DVE/ACT; PSUM budget: proj+out-proj 2 banks, paired
    scores 2x2 banks, attention accumulators 2 banks
"""

import sys

if "/opt/trn_rl_repo" not in sys.path:
    sys.path.insert(0, "/opt/trn_rl_repo")

import numpy as np

import concourse.bass as bass  # noqa: F401  (engine namespaces live on nc)
import concourse.tile as tile
from concourse import bacc, bass_isa, mybir
from concourse.bass_utils import run_bass_kernel_spmd

F32 = mybir.dt.float32
FP8 = mybir.dt.float8e4
BF16 = mybir.dt.bfloat16
AF = mybir.ActivationFunctionType
ALU = mybir.AluOpType

B, S, E = 2, 2048, 2048
H, D = 16, 128
NCORES = 8
HPC = 4                    # heads per core
M = HPC * D                # local qkv channels per core = 512
EO = E // 128              # 16 contraction chunks
XT = 512                   # token-tile width for projections
NT = S // XT               # 4 token tiles
QT = 512                   # q-tile width for attention
NQT = S // QT              # 4 q-tiles
ET = 512                   # e-tile width for out-projection
SCALE = 1.0 / float(np.sqrt(D))
MASK_BIAS = -30.0


def build_nc():
    nc = bacc.Bacc(trn_type="TRN2", target_bir_lowering=False, num_swdge_queues=4)

    xT = nc.declare_dram_parameter("xT", [E, S], BF16, isOutput=False)
    x8 = nc.declare_dram_parameter("x8", [E, S], FP8, isOutput=False)
    wq = nc.declare_dram_parameter("wq", [E, M], FP8, isOutput=False)
    wk = nc.declare_dram_parameter("wk", [E, M], FP8, isOutput=False)
    wv = nc.declare_dram_parameter("wv", [E, M], BF16, isOutput=False)
    wo = nc.declare_dram_parameter("wo", [M, E], BF16, isOutput=False)
    bq = nc.declare_dram_parameter("bq", [128, HPC], F32, isOutput=False)
    bk = nc.declare_dram_parameter("bk", [128, HPC], F32, isOutput=False)
    tb = nc.declare_dram_parameter("tb", [128, 128], F32, isOutput=False)
    o = nc.declare_dram_parameter("o", [S, E], BF16, isOutput=True)

    with tile.TileContext(nc) as tc:
        _body(tc, nc, xT, x8, wq, wk, wv, wo, bq, bk, tb, o)
    nc.compile()
    return nc


def _body(tc, nc, xT, x8, wq, wk, wv, wo, bq, bk, tb, o):
    from contextlib import ExitStack

    ctx = ExitStack()
    with ctx:
        # PSUM pools first so the paired score tile lands bank-aligned.
        psS = ctx.enter_context(tc.tile_pool(name="psS", bufs=2, space="PSUM"))
        psA = ctx.enter_context(tc.tile_pool(name="psA", bufs=2, space="PSUM"))
        psU = ctx.enter_context(tc.tile_pool(name="psU", bufs=2, space="PSUM"))
        wpool = ctx.enter_context(tc.tile_pool(name="w", bufs=1))
        xpool = ctx.enter_context(tc.tile_pool(name="x", bufs=2))
        x8pool = ctx.enter_context(tc.tile_pool(name="x8", bufs=2))
        qkv = ctx.enter_context(tc.tile_pool(name="qkv", bufs=1))
        otp = ctx.enter_context(tc.tile_pool(name="ot", bufs=1))
        epool = ctx.enter_context(tc.tile_pool(name="e", bufs=6))
        eap = ctx.enter_context(tc.tile_pool(name="ea", bufs=4))
        dpool = ctx.enter_context(tc.tile_pool(name="dn", bufs=3))
        osp = ctx.enter_context(tc.tile_pool(name="os", bufs=8))

        # ---- weights / constants ----
        wq_sb = wpool.tile([128, EO, M], FP8, tag="wq")
        wk_sb = wpool.tile([128, EO, M], FP8, tag="wk")
        wv_sb = wpool.tile([128, EO, M], BF16, tag="wv")
        wo_sb = wpool.tile([128, HPC, E], BF16, tag="wo")

        # Warm the PE (HAM clock gate) with tiny matmuls while x0/weights
        # stream in; the warm tiles come from memsets so they have no DMA
        # dependency.
        warm_l = wpool.tile([128, 1], BF16, tag="wl")
        warm_r = wpool.tile([128, 512], BF16, tag="wr")
        nc.vector.memset(warm_l[:], 0.0)
        nc.vector.memset(warm_r[:], 0.0)
        warm = psS.tile([128, 2, 512], F32, tag="sc")
        for _ in range(18):
            nc.tensor.matmul(warm[:1, 0, :], warm_l[:], warm_r[:],
                             start=True, stop=True)

        # x tile 0 and the first head-pair's wq/wk stream in first so the
        # first projection groups start as soon as possible; wv is needed by
        # the v-projection at the end of tile 0, wo only at the first
        # out-projection.
        _wqr = wq.rearrange("(eo p) m -> p eo m", p=128)
        _wkr = wk.rearrange("(eo p) m -> p eo m", p=128)
        _xr = xT.rearrange("(eo p) s -> p eo s", p=128)
        _x8r = x8.rearrange("(eo p) s -> p eo s", p=128)
        x8_first = x8pool.tile([128, EO, 512], FP8, tag="x8")
        x_first = xpool.tile([128, EO, 512], BF16, tag="x")
        nc.sync.dma_start(x8_first[:], _x8r[:, :, 0:512])
        nc.sync.dma_start(wq_sb[:], _wqr[:])
        nc.gpsimd.dma_start(wk_sb[:], _wkr[:])
        nc.gpsimd.dma_start(x_first[:], _xr[:, :, 0:512])
        nc.gpsimd.dma_start(wv_sb[:], wv.rearrange("(eo p) m -> p eo m", p=128))
        x8_second = x8pool.tile([128, EO, 512], FP8, tag="x8")
        nc.gpsimd.dma_start(x8_second[:], _x8r[:, :, 512:1024])
        x_second = xpool.tile([128, EO, 512], BF16, tag="x")
        nc.gpsimd.dma_start(x_second[:], _xr[:, :, 512:1024])
        nc.sync.dma_start(wo_sb[:], wo.rearrange("(h p) e -> p h e", p=128))

        bq_sb = wpool.tile([128, HPC], F32, tag="bq")
        bk_sb = wpool.tile([128, HPC], F32, tag="bk")
        tb_sb = wpool.tile([128, 128], F32, tag="tb")
        nc.sync.dma_start(bq_sb[:], bq[:])
        nc.sync.dma_start(bk_sb[:], bk[:])
        nc.sync.dma_start(tb_sb[:], tb[:])

        qT_sb = qkv.tile([128, HPC, S], BF16, tag="qT")
        kT_sb = qkv.tile([128, HPC, S], BF16, tag="kT")
        v_sb = qkv.tile([128, S // 128, M], BF16, tag="v")
        oT_sb = otp.tile([128, HPC, S], BF16, tag="oT")

        def proj_group(off, wdt, x8_t, h, w_sb, dst, bias, scl):
            # fp8 DoubleRow: each matmul contracts 256 channels (2 eo-pairs
            # packed on the partition dim); weights are host-prescaled by 64
            # and the evac scale divides it back out
            ps = psA.tile([128, 512], F32, tag="qkv")
            for j in range(EO // 2):
                nc.tensor.matmul(
                    ps[:, :wdt],
                    w_sb[:, 2 * j:2 * j + 2, h * D:(h + 1) * D],
                    x8_t[:, 2 * j:2 * j + 2, :wdt],
                    start=(j == 0),
                    stop=(j == EO // 2 - 1),
                    perf_mode=mybir.MatmulPerfMode.DoubleRow,
                )
            nc.scalar.activation(
                dst[:, h, off:off + wdt],
                ps[:, :wdt],
                AF.Identity,
                bias=bias[:, h:h + 1],
                scale=scl,
            )

        def proj_tile(off, wdt, x_t, x8_t):
            # head-pair-major order so the first groups only need the first
            # half of wq/wk
            for hp in (0, 1):
                for h in (2 * hp, 2 * hp + 1):
                    proj_group(off, wdt, x8_t, h, wq_sb, qT_sb, bq_sb, SCALE / 64.0)
                for h in (2 * hp, 2 * hp + 1):
                    proj_group(off, wdt, x8_t, h, wk_sb, kT_sb, bk_sb, 1.0 / 64.0)
            for st in range(wdt // 128):
                ps = psA.tile([128, 512], F32, tag="qkv")
                for eo in range(EO):
                    nc.tensor.matmul(
                        ps[:],
                        x_t[:, eo, st * 128:(st + 1) * 128],
                        wv_sb[:, eo, :],
                        start=(eo == 0),
                        stop=(eo == EO - 1),
                    )
                nc.vector.tensor_copy(v_sb[:, off // 128 + st, :], ps[:])

        def attn_pair(hp, qt):
            ha, hb = 2 * hp, 2 * hp + 1
            q_a = qT_sb[:, ha, qt * QT:(qt + 1) * QT]
            q_b = qT_sb[:, hb, qt * QT:(qt + 1) * QT]
            ut_a = psU.tile([128, 512], F32, tag="ut")
            ut_b = psU.tile([128, 512], F32, tag="ut")
            ea_a = eap.tile([128, 512], BF16, tag="ea")
            ea_b = eap.tile([128, 512], BF16, tag="ea")
            nkt = (qt + 1) * (QT // 128)
            for kt in range(nkt):
                jj = kt - qt * (QT // 128)
                # columns < jj*128 of this k-tile's block are causally
                # masked; diagonal tiles come last in the k-loop, so
                # accumulating only the valid sub-range is exact.
                lo = max(jj, 0) * 128
                sc = psS.tile([128, 2, 512], F32, tag="sc")
                for j, q_h, hh in ((0, q_a, ha), (1, q_b, hb)):
                    nc.tensor.matmul(
                        sc[:, j, lo:],
                        kT_sb[:, hh, kt * 128:(kt + 1) * 128],
                        q_h[:, lo:],
                        start=True,
                        stop=True,
                    )
                if jj >= 0:
                    nc.vector.tensor_tensor(
                        sc[:, 0, lo:lo + 128], sc[:, 0, lo:lo + 128],
                        tb_sb[:], ALU.add,
                    )
                    nc.vector.tensor_tensor(
                        sc[:, 1, lo:lo + 128], sc[:, 1, lo:lo + 128],
                        tb_sb[:], ALU.add,
                    )
                e3 = epool.tile([128, 2, 512], BF16, tag="e")
                nc.scalar.activation(e3[:, :, lo:], sc[:, :, lo:], AF.Exp)
                if kt == 0:
                    nc.vector.tensor_copy(ea_a[:], e3[:, 0])
                    nc.vector.tensor_copy(ea_b[:], e3[:, 1])
                else:
                    nc.vector.tensor_tensor(
                        ea_a[:, lo:], ea_a[:, lo:], e3[:, 0, lo:], ALU.add
                    )
                    nc.vector.tensor_tensor(
                        ea_b[:, lo:], ea_b[:, lo:], e3[:, 1, lo:], ALU.add
                    )
                nc.tensor.matmul(
                    ut_a[:, lo:],
                    v_sb[:, kt, ha * D:(ha + 1) * D],
                    e3[:, 0, lo:],
                    start=(kt == 0),
                    stop=(kt == nkt - 1),
                )
                nc.tensor.matmul(
                    ut_b[:, lo:],
                    v_sb[:, kt, hb * D:(hb + 1) * D],
                    e3[:, 1, lo:],
                    start=(kt == 0),
                    stop=(kt == nkt - 1),
                )
            for ea_h, ut_h, h in ((ea_a, ut_a, ha), (ea_b, ut_b, hb)):
                dn = dpool.tile([128, 512], F32, tag="dn")
                nc.gpsimd.partition_all_reduce(
                    dn[:], ea_h[:], 128, bass_isa.ReduceOp.add
                )
                rcp = dpool.tile([128, 512], F32, tag="rcp")
                nc.vector.reciprocal(rcp[:], dn[:])
                nc.vector.tensor_tensor(
                    oT_sb[:, h, qt * QT:(qt + 1) * QT], ut_h[:], rcp[:],
                    ALU.mult,
                )

        def out_proj(qt, tail=False, half=None):
            qrange = range(QT // 128)
            if half == 0:
                qrange = range(0, QT // 256)
            elif half == 1:
                qrange = range(QT // 256, QT // 128)
            for qi4 in qrange:
                qi = qt * (QT // 128) + qi4
                for et in range(E // ET):
                    ps = psA.tile([128, 512], F32, tag="qkv")
                    for h in range(HPC):
                        nc.tensor.matmul(
                            ps[:],
                            oT_sb[:, h, qi * 128:(qi + 1) * 128],
                            wo_sb[:, h, et * ET:(et + 1) * ET],
                            start=(h == 0),
                            stop=(h == HPC - 1),
                        )
                    last = qi4 == QT // 128 - 1 and et == E // ET - 1
                    if tail and last:
                        for sl in range(2):
                            osb = osp.tile([128, 256], BF16, tag="osbt")
                            if sl == 0:
                                nc.vector.tensor_copy(
                                    osb[:], ps[:, 0:256])
                            else:
                                nc.scalar.copy(osb[:], ps[:, 256:512])
                            nc.sync.dma_start(
                                o[qi * 128:(qi + 1) * 128,
                                  et * ET + sl * 256:et * ET + (sl + 1) * 256],
                                osb[:],
                            )
                        continue
                    osb = osp.tile([128, 512], BF16, tag="osb")
                    if (qi * (E // ET) + et) % 2 == 0:
                        nc.vector.tensor_copy(osb[:], ps[:])
                    else:
                        nc.scalar.copy(osb[:], ps[:])
                    nc.sync.dma_start(
                        o[qi * 128:(qi + 1) * 128, et * ET:(et + 1) * ET],
                        osb[:],
                    )

        # Emission order = scheduler priority.  attn(qt) only needs
        # projection tiles 0..qt (causal), so attention interleaves into the
        # projection phase: each ACT-paced attention stretch has the next
        # projection tile's dense PE work as filler, and the late q-tiles
        # get the (deliberately delayed) out_proj groups as filler.
        tiles = [(0, 512), (512, 512), (1024, 512), (1536, 512)]
        attn_after = {0: 0, 1: 1, 2: 2, 3: 3}
        for i, (off, wdt) in enumerate(tiles):
            if i == 0:
                x_t, x8_t = x_first, x8_first
            elif i == 1:
                x_t, x8_t = x_second, x8_second
            else:
                x8_t = x8pool.tile([128, EO, 512], FP8, tag="x8")
                nc.gpsimd.dma_start(x8_t[:, :, :wdt], _x8r[:, :, off:off + wdt])
                x_t = xpool.tile([128, EO, 512], BF16, tag="x")
                nc.gpsimd.dma_start(x_t[:, :, :wdt], _xr[:, :, off:off + wdt])
            proj_tile(off, wdt, x_t, x8_t)
            if i in attn_after:
                qt = attn_after[i]
                attn_pair(0, qt)
                if qt >= 2:
                    out_proj(qt - 2, half=0)
                attn_pair(1, qt)
                if qt >= 2:
                    out_proj(qt - 2, half=1)
                if qt == NQT - 1:
                    out_proj(NQT - 2)
        out_proj(NQT - 1)


_NC_CACHE = None


def _get_nc():
    global _NC_CACHE
    if _NC_CACHE is None:
        _NC_CACHE = build_nc()
    return _NC_CACHE


def _prep_inputs(x, Wq, bq, Wk, bk, Wv, bv, Wo, bo):
    import ml_dtypes

    BF = ml_dtypes.bfloat16
    x = np.asarray(x, dtype=np.float32)
    tb_np = np.where(
        np.arange(128)[:, None] <= np.arange(128)[None, :], 0.0, MASK_BIAS
    ).astype(np.float32)
    F8 = ml_dtypes.float8_e4m3fn
    xT_b = [
        np.ascontiguousarray(x[b].T.astype(BF)) for b in range(B)
    ]
    x8_b = [
        np.ascontiguousarray(x[b].T.astype(F8)) for b in range(B)
    ]
    Wq = np.asarray(Wq, dtype=np.float32)
    Wk = np.asarray(Wk, dtype=np.float32)
    Wv = np.asarray(Wv, dtype=np.float32)
    Wo = np.asarray(Wo, dtype=np.float32)
    bq = np.asarray(bq, dtype=np.float32)
    bk = np.asarray(bk, dtype=np.float32)
    in_maps = []
    for c in range(NCORES):
        b = c // 4
        g = c % 4
        sl = slice(g * M, (g + 1) * M)
        in_maps.append({
            "xT": xT_b[b],
            "x8": x8_b[b],
            "wq": np.ascontiguousarray((Wq[sl, :].T * 64.0).astype(F8)),
            "wk": np.ascontiguousarray((Wk[sl, :].T * 64.0).astype(F8)),
            "wv": np.ascontiguousarray(Wv[sl, :].T.astype(BF)),
            "wo": np.ascontiguousarray(Wo[:, sl].T.astype(BF)),
            "bq": np.ascontiguousarray(
                (bq[sl] * SCALE).reshape(HPC, 128).T.astype(np.float32)
            ),
            "bk": np.ascontiguousarray(
                bk[sl].reshape(HPC, 128).T.astype(np.float32)
            ),
            "tb": tb_np,
        })
    return in_maps


def run(inputs, trace=False):
    in_maps = _prep_inputs(
        inputs["x"], inputs["Wq"], inputs["bq"], inputs["Wk"], inputs["bk"],
        inputs["Wv"], inputs["bv"], inputs["Wo"], inputs["bo"],
    )
    nc = _get_nc()
    res = run_bass_kernel_spmd(nc, in_maps, list(range(NCORES)), trace=trace)
    acc = np.zeros((B, S, E), dtype=np.float64)
    for c, r in enumerate(res.results):
        acc[c // 4] += np.asarray(r["o"]).astype(np.float64)
    acc += np.asarray(inputs["bo"], dtype=np.float64)[None, None, :]
    acc += (np.asarray(inputs["bv"], dtype=np.float64)
            @ np.asarray(inputs["Wo"], dtype=np.float64).T)[None, None, :]
    return acc.astype(np.float32), res


def kernel(**inputs):
    out, _ = run(inputs, trace=False)
    return out
